# revision 45
# baseline (speedup 1.0000x reference)
"""Fused conv-attention kernel for Trainium2, sharded over 8 NeuronCores.

Reference computation (B=2, H=12, L=T=1024, D=64, FEA=3, DIM=768):
    scores = concat([s0,s1,s2], ch)            # [b, 36, l, t]
    fused  = einsum('bclt,oc->bolt', scores, fuse_w) + fuse_b
    attn   = softmax(fused, axis=-1)
    x      = einsum('bhlt,bhtd->bhld', attn, v)
    y      = merge_heads(x) @ proj_w.T + proj_b  # [b, l, 768]

Sharding: fully data-parallel over (b, l-block): core k handles b=k//4 and
l-rows [256*(k%4), 256*(k%4)+256).  Every op is local; no collectives.

All inputs are downcast + repacked to fp16 on the host into exactly the
SBUF layouts the kernel wants.  Matmuls run fp16, proj bias is folded into
PSUM via a ones-row matmul, and the attn@V + proj tail is chunked (2 x 128
l-rows) so it overlaps the score streaming of the next chunk.

The attn^T transposes run on the DMA XBAR (dma_start(transpose=True),
16x128 tiles), writing softmax rows straight into the attnT gather layout:
out[p, k, m] = in[m, k*128+p].  The scheduler gives XBAR-mode DMA an
exclusive window on the DMA channel (hw deadlock workaround), so the
transposes are batched into ONE window per chunk (GQ=16) to pay the
passthrough<->transpose drain only 4 times, and score streaming runs from
a 6-deep tile queue so conv/exp keep executing through each window.

Score streaming uses per-group [96, 3072] tiles (6KB/partition-line HWDGE
descriptors, the empirically fastest shape: ~16.3GB/s/engine vs 11.9 at
24KB) issued from the Sync ring; init loads + output stores use the Scalar
ring; ScalarE itself only runs exp (the XBAR triggers would otherwise
block it ~4us each).

attn@V runs 2 heads concurrently in disjoint PE column groups
(tile_position=(0,0)/(0,64)) accumulating into one [128, 128] PSUM bank,
which also makes the PSUM->SBUF cast per head-pair a single [128, 128]
tensor_copy whose partition layout exactly matches the row-parallel proj
lhsT blocks.
"""

import os
import sys

import numpy as np

sys.path.insert(0, "/opt/trn_rl_repo")

B, H, L, T, D = 2, 12, 1024, 1024, 64
DIM = H * D  # 768
NCORES = 8
LC = L * B // NCORES  # 256 l-rows per core
G = 8  # l-rows per conv group
NG = LC // G  # 32 groups
KM = 12 * G  # 96: conv matmul K and M
NTT = T // 128  # 8 t-tiles
# uneven l-chunks: each chunk's XBAR window hides under the next chunk's
# compute; the last (smallest) chunk minimizes the exposed window + tail
CS = [12, 12, 8]  # groups per chunk
NCH = len(CS)
CO = [sum(CS[:i]) for i in range(NCH)]  # group offsets
LCHS = [cs * G for cs in CS]  # l-rows per chunk (96, 96, 64)
LO = [co * G for co in CO]  # l-row offsets

_CACHE = {}


def _build_nc():
    import concourse.bacc as bacc
    import concourse.bass as bass
    import concourse.mybir as mybir
    import concourse.tile as tile
    from concourse.masks import make_identity
    from contextlib import ExitStack

    f32 = mybir.dt.float32
    f16 = mybir.dt.float16
    f8 = mybir.dt.float8e4

    nc = bacc.Bacc(
        "TRN2", target_bir_lowering=False, debug=False, enable_asserts=False
    )

    # host-packed inputs (scores fp8, rest fp16 except biases/output)
    # scores: fp8e4m3 (error budget: adds ~1.2e-2 absmax-rel vs the 2e-2
    # gate; conv weights STAY fp16 — mixed fp16-stationary x fp8-moving
    # matmul verified exact on HW), packed 2 groups per tile so partition
    # lines stay at the empirically best 6KB descriptor size.
    sp_in = nc.dram_tensor("sp", [NG // 2, KM, 6 * T], f8, kind="ExternalInput").ap()
    vp_in = nc.dram_tensor("vp", [128, H * NTT * D], f16, kind="ExternalInput").ap()
    # conv weights padded to M=128 (32 dummy output rows) so the compiler
    # enables Fast Weight Load (NumWeights==128) on every conv matmul
    wp_in = nc.dram_tensor("wp", [KM, 3 * 128], f16, kind="ExternalInput").ap()
    b_in = nc.dram_tensor("b96", [KM, 1], f32, kind="ExternalInput").ap()
    pw_in = nc.dram_tensor("pwp", [128, 6 * DIM], f16, kind="ExternalInput").ap()
    pb_in = nc.dram_tensor("pbr", [1, DIM], f16, kind="ExternalInput").ap()
    out_d = nc.dram_tensor("out", [LC, DIM], f16, kind="ExternalOutput").ap()

    with tile.TileContext(nc) as tc, ExitStack() as ctx:
        # ---- persistent SBUF ----
        singles = ctx.enter_context(tc.tile_pool(name="singles", bufs=1))
        ones1 = singles.tile([1, 128], f16)
        nc.vector.memset(ones1[:], 1.0)
        # dummy activation: pulls the ~2.7us exp ACT_TABLE_LOAD to t=0,
        # off the first real exp's critical path
        dum = singles.tile([1, 1], f32)
        nc.vector.memset(dum[:], 0.0)
        dum2 = singles.tile([1, 1], f32)
        nc.scalar.activation(dum2[:], dum[:], mybir.ActivationFunctionType.Exp)
        # only what the first conv+exp needs loads up front; the big vsb/pw
        # loads are issued later (phase 1) so they don't delay the first
        # score tiles on the shared SDMA engines
        wsb = singles.tile([KM, 3 * 128], f16)
        nc.sync.dma_start(wsb[:], wp_in)
        b96 = singles.tile([KM, 1], f32)
        nc.sync.dma_start(b96[:], b_in)
        pbr = singles.tile([1, DIM], f16)
        nc.scalar.dma_start(pbr[:], pb_in)
        vsb = singles.tile([128, H * NTT * D], f16)  # [t-part, h*512 + tt*64 + d]
        pw = singles.tile([128, 6 * DIM], f16)  # [i-part, ki*768 + o]
        # per-chunk attn^T: [t%128-part(128), gr*(NTT*96) + tt*96 + (h*8+lg)]
        # holds attn[l=LO[c] + gr*8 + lg, t=tt*128+p] for head h.
        # gr-major so the chunk's XBAR transpose writes one contiguous block.
        attnT = [
            singles.tile(
                [128, CS[c] * NTT * KM], f16, tag=f"attnT{c}", name=f"attnT{c}"
            )
            for c in range(NCH)
        ]
        # per-chunk x^T for proj: [i%128 part, (i//128)*LCHS[c] + l]
        xT = [
            singles.tile([128, 6 * LCHS[c]], f16, tag=f"xT{c}", name=f"xT{c}")
            for c in range(NCH)
        ]

        with ExitStack() as p1:
            spool = p1.enter_context(tc.tile_pool(name="scores", bufs=6))
            fpsum = p1.enter_context(tc.tile_pool(name="fpsum", bufs=2, space="PSUM"))
            npool = p1.enter_context(tc.tile_pool(name="norm", bufs=2))
            zpool = p1.enter_context(tc.tile_pool(name="z", bufs=4))
            xpsum = p1.enter_context(tc.tile_pool(name="xpsum", bufs=2, space="PSUM"))
            ppsum = p1.enter_context(tc.tile_pool(name="ppsum", bufs=1, space="PSUM"))
            ypool = p1.enter_context(tc.tile_pool(name="y", bufs=2))

            # Score-DMA issue runs LOOKAHEAD pairs ahead of consumption so
            # the next chunk's tiles are emitted BEFORE each chunk's xbar
            # trigger: otherwise they queue behind the ~17us exclusive
            # transpose window (in the Sync FIFO and in the scheduler's DMA
            # serialization order) and conv starves during the window.
            LA_PAIRS = 4  # 8 groups of lookahead
            st2_tiles = {}

            def issue_sp(p):
                if p >= NG // 2:
                    return
                st2 = spool.tile([KM, 6 * T], f8, tag="st2", name=f"st2_{p}")
                nc.sync.dma_start(st2[:], sp_in[p])
                st2_tiles[p] = st2

            for p in range(LA_PAIRS):
                issue_sp(p)

            for c in range(NCH):
                # ---- phase 1: conv + softmax, per pair of groups ----
                # (j-outer over the pair so identical conv weights are
                # consecutive on the PE queue)
                for gp in range(CS[c] // 2):
                    kp = CO[c] // 2 + gp  # global pair index
                    g0 = CO[c] + 2 * gp
                    issue_sp(kp + LA_PAIRS)
                    st2 = st2_tiles.pop(kp)
                    fps = []
                    for gi in range(2):
                        fps.append(
                            fpsum.tile([128, T], f32, tag="fp", name=f"fp_{g0 + gi}")
                        )
                    if kp == 5:
                        # big init loads, once the pipeline is rolling
                        nc.scalar.dma_start(vsb[:], vp_in)
                        nc.scalar.dma_start(pw[:], pw_in)
                    for j in range(3):
                        for gi in range(2):
                            for th in range(2):
                                nc.tensor.matmul(
                                    fps[gi][:, th * 512 : (th + 1) * 512],
                                    wsb[:, j * 128 : (j + 1) * 128],
                                    st2[
                                        :,
                                        gi * 3 * T
                                        + j * T
                                        + th * 512 : gi * 3 * T
                                        + j * T
                                        + (th + 1) * 512,
                                    ],
                                    start=(j == 0),
                                    stop=(j == 2),
                                )
                    for gi in range(2):
                        g = g0 + gi
                        q = g - CO[c]  # position in this chunk's window
                        if q == 0:
                            etn = npool.tile(
                                [KM, CS[c] * T], f16, tag="etn", name=f"etn_{g}"
                            )
                        # exp (+bias) straight into the window tile, with
                        # row-sum accumulation; then normalize in place
                        ecol = etn[:, q * T : (q + 1) * T]
                        zt = zpool.tile([KM, 1], f32, tag="zt")
                        nc.scalar.activation(
                            ecol,
                            fps[gi][0:KM, :],
                            mybir.ActivationFunctionType.Exp,
                            bias=b96[:],
                            accum_out=zt[:],
                        )
                        zi = zpool.tile([KM, 1], f32, tag="zi")
                        nc.vector.reciprocal(zi[:], zt[:])
                        nc.vector.tensor_scalar_mul(ecol, ecol, zi[:])
                        if q == CS[c] - 1:
                            # DMA XBAR transpose of the chunk's window:
                            # out[p, k, m] = etn[m, k*128+p], k = (gr, tt)
                            # (out must stay 3D: middle dim extends partition)
                            dst = attnT[c][:].rearrange("p (k m) -> p k m", m=KM)[
                                :, 0 : CS[c] * NTT, :
                            ]
                            nc.sync.dma_start(dst, etn[:], transpose=True)

                # ---- phase 2: attn @ V -> x^T, 2 heads per PSUM bank ----
                LCH = LCHS[c]
                atv = attnT[c][:].rearrange(
                    "p (gr tt h lg) -> p gr tt h lg", gr=CS[c], tt=NTT, h=H
                )
                for hp in range(H // 2):
                    h0, h1 = 2 * hp, 2 * hp + 1
                    xp = xpsum.tile([128, LCH], f32, tag="xp", name=f"xp_{c}_{hp}")
                    for tt in range(NTT):
                        nc.tensor.matmul(
                            xp[0:64, :],
                            vsb[:, h0 * 512 + tt * D : h0 * 512 + (tt + 1) * D],
                            atv[:, :, tt, h0, :],
                            start=(tt == 0),
                            stop=(tt == NTT - 1),
                            tile_position=(0, 0),
                        )
                        nc.tensor.matmul(
                            xp[64:128, :],
                            vsb[:, h1 * 512 + tt * D : h1 * 512 + (tt + 1) * D],
                            atv[:, :, tt, h1, :],
                            start=(tt == 0),
                            stop=(tt == NTT - 1),
                            tile_position=(0, 64),
                        )
                    nc.vector.tensor_copy(
                        xT[c][:, hp * LCH : (hp + 1) * LCH], xp[:]
                    )

                # ---- phase 3: proj (+bias via ones-row matmul) -> out ----
                pp = ppsum.tile([128, DIM], f32, tag="pp", name=f"pp_{c}")
                nc.tensor.matmul(
                    pp[0:LCH, 0:512],
                    ones1[:, 0:LCH],
                    pbr[:, 0:512],
                    start=True,
                    stop=False,
                )
                nc.tensor.matmul(
                    pp[0:LCH, 512:768],
                    ones1[:, 0:LCH],
                    pbr[:, 512:768],
                    start=True,
                    stop=False,
                )
                for ki in range(6):
                    lhs = xT[c][:, ki * LCH : (ki + 1) * LCH]
                    nc.tensor.matmul(
                        pp[0:LCH, 0:512],
                        lhs,
                        pw[:, ki * DIM : ki * DIM + 512],
                        start=False,
                        stop=(ki == 5),
                    )
                    nc.tensor.matmul(
                        pp[0:LCH, 512:768],
                        lhs,
                        pw[:, ki * DIM + 512 : ki * DIM + DIM],
                        start=False,
                        stop=(ki == 5),
                    )
                yt = ypool.tile([128, DIM], f16, tag="yt", name=f"yt_{c}")
                nc.vector.tensor_copy(yt[0:LCH, :], pp[0:LCH, :])
                nc.scalar.dma_start(out_d[LO[c] : LO[c] + LCH, :], yt[0:LCH, :])

    nc.compile()
    return nc


def _host_prep(s0, s1, s2, v, fuse_w, fuse_b, proj_w, proj_b):
    """Build per-core input maps (fp16 repack in exactly-SBUF layouts)."""
    s0 = np.asarray(s0, dtype=np.float32)
    s1 = np.asarray(s1, dtype=np.float32)
    s2 = np.asarray(s2, dtype=np.float32)
    v = np.asarray(v, dtype=np.float32)
    fuse_w = np.asarray(fuse_w, dtype=np.float32)
    fuse_b = np.asarray(fuse_b, dtype=np.float32)
    proj_w = np.asarray(proj_w, dtype=np.float32)
    proj_b = np.asarray(proj_b, dtype=np.float32)

    # block-diag conv weights: wp[k=(lg,c), j, m=(o,lg)] = fuse_w[o, 12j+c]
    # (padded to M=128 columns per j-block to enable FWL)
    wp = np.zeros((KM, 3, 128), dtype=np.float16)
    for j in range(3):
        blk = fuse_w[:, 12 * j : 12 * (j + 1)].T.astype(np.float16)  # [c, o]
        for lg in range(G):
            wp[lg * 12 : (lg + 1) * 12, j, lg : KM : G] = blk
    wp = wp.reshape(KM, 3 * 128)
    b96 = np.repeat(fuse_b, G).astype(np.float32).reshape(KM, 1)  # p = o*G+lg
    # proj weight chunks: pwp[p, ki, o] = proj_w[o, ki*128+p]
    pwp = (
        np.ascontiguousarray(proj_w.T.reshape(6, 128, DIM))
        .transpose(1, 0, 2)
        .reshape(128, 6 * DIM)
        .astype(np.float16)
    )
    pbr = proj_b.reshape(1, DIM).astype(np.float16)

    in_maps = []
    for k in range(NCORES):
        b = k // (NCORES // B)
        l0 = (k % (NCORES // B)) * LC
        # scores: sp[g, p=(lg*12+c), j, t] = s_j[b, c, l0+g*8+lg, t]
        sj = np.stack(
            [s[b, :, l0 : l0 + LC, :] for s in (s0, s1, s2)], axis=0
        )  # [3, 12, 256, 1024]
        import ml_dtypes

        sp = (
            sj.reshape(3, 12, NG, G, T)
            .transpose(2, 3, 1, 0, 4)  # [g, lg, c, j, t]
            .reshape(NG, KM, 3 * T)
            .astype(ml_dtypes.float8_e4m3fn)
            # 2 groups per tile: [p_, part, (gi, j, t)] -> 6KB fp8 lines
            .reshape(NG // 2, 2, KM, 3 * T)
            .transpose(0, 2, 1, 3)
            .reshape(NG // 2, KM, 6 * T)
        )
        # v: vp[p, h, tt, d] = v[b, h, tt*128+p, d]
        vp = (
            np.ascontiguousarray(v[b])  # [12, 1024, 64]
            .reshape(H, NTT, 128, D)
            .transpose(2, 0, 1, 3)
            .reshape(128, H * NTT * D)
            .astype(np.float16)
        )
        m = {
            "sp": np.ascontiguousarray(sp),
            "vp": np.ascontiguousarray(vp),
            "wp": wp,
            "b96": b96,
            "pwp": pwp,
            "pbr": pbr,
        }
        in_maps.append(m)
    return in_maps


def _install_ntff_hook():
    """Provide antenv.axon_hooks (absent in this image) so trace=True works."""
    try:
        from antenv import axon_hooks  # noqa: F401

        return True
    except ImportError:
        pass
    try:
        import types
        import ctypes
        import contextlib
        import antenv

        so_path = "/opt/axon/libaxon_pjrt.so"
        if not os.path.exists(so_path):
            return False
        lib = ctypes.CDLL(so_path)
        if not hasattr(lib, "axon_start_nrt_profile"):
            return False
        lib.axon_start_nrt_profile.argtypes = [
            ctypes.POINTER(ctypes.c_int64),
            ctypes.c_size_t,
        ]
        lib.axon_start_nrt_profile.restype = ctypes.c_int64
        lib.axon_stop_nrt_profile.argtypes = [ctypes.c_char_p]
        lib.axon_stop_nrt_profile.restype = ctypes.c_int64

        @contextlib.contextmanager
        def _hook(output_dir, device_ids):
            import jax

            jax.devices()
            if device_ids:
                ids = (ctypes.c_int64 * len(device_ids))(*device_ids)
                rc = lib.axon_start_nrt_profile(ids, len(device_ids))
            else:
                rc = lib.axon_start_nrt_profile(None, 0)
            if rc != 0:
                raise RuntimeError(f"axon_start_nrt_profile rc={rc}")
            try:
                yield
            finally:
                n = lib.axon_stop_nrt_profile(str(output_dir).encode())
                print(f"ntff profile: {n} file(s) -> {output_dir}", file=sys.stderr)

        mod = types.ModuleType("antenv.axon_hooks")
        _h = {"hook": _hook}
        mod.set_axon_ntff_profile_hook = lambda h: _h.__setitem__("hook", h)
        mod.get_axon_ntff_profile_hook = lambda: _h["hook"]
        sys.modules["antenv.axon_hooks"] = mod
        antenv.axon_hooks = mod
        return True
    except Exception as e:  # degrade to untraced
        print("ntff hook install failed:", e, file=sys.stderr)
        return False


def kernel(s0, s1, s2, v, fuse_w, fuse_b, proj_w, proj_b, _trace=False):
    from concourse import bass_utils
    from concourse.bass_utils import run_bass_kernel_spmd

    if "nc" not in _CACHE:
        _CACHE["nc"] = _build_nc()
    nc = _CACHE["nc"]

    in_maps = _host_prep(s0, s1, s2, v, fuse_w, fuse_b, proj_w, proj_b)
    if _trace:
        _trace = _install_ntff_hook()
        bass_utils.upload_artifacts = lambda tmpdir: f"local:{tmpdir}"
    tmpdir = None
    if _trace:
        import tempfile

        tmpdir = tempfile.mkdtemp(prefix="bass_trace_")
        _CACHE["trace_dir"] = tmpdir
    try:
        res = run_bass_kernel_spmd(
            nc, in_maps, core_ids=list(range(NCORES)), trace=_trace, tmpdir=tmpdir
        )
    except Exception:
        if not _trace:
            raise
        import traceback

        traceback.print_exc()
        print("trace run failed; retrying untraced", file=sys.stderr)
        res = run_bass_kernel_spmd(nc, in_maps, core_ids=list(range(NCORES)))
    _CACHE["last_exec_time_ns"] = res.exec_time_ns
    _CACHE["last_results"] = res

    out = np.empty((B, L, DIM), dtype=np.float32)
    for k in range(NCORES):
        b = k // (NCORES // B)
        l0 = (k % (NCORES // B)) * LC
        out[b, l0 : l0 + LC, :] = res.results[k]["out"].astype(np.float32)
    return out


# revision 48
# speedup vs baseline: 1.1639x; 1.1639x over previous
"""Fused conv-attention kernel for Trainium2, sharded over 8 NeuronCores.

Reference computation (B=2, H=12, L=T=1024, D=64, FEA=3, DIM=768):
    scores = concat([s0,s1,s2], ch)            # [b, 36, l, t]
    fused  = einsum('bclt,oc->bolt', scores, fuse_w) + fuse_b
    attn   = softmax(fused, axis=-1)
    x      = einsum('bhlt,bhtd->bhld', attn, v)
    y      = merge_heads(x) @ proj_w.T + proj_b  # [b, l, 768]

Sharding: fully data-parallel over (b, l-block): core k handles b=k//4 and
l-rows [256*(k%4), 256*(k%4)+256).  Every op is local; no collectives.

Final design (186us baseline -> ~109us):

* Scores stream as fp8e4m3 (9.4MB/core instead of 18.9 fp16; adds ~1.5e-2
  absmax-rel error vs the 2e-2 gate, deterministic).  Conv weights stay
  fp16 -- mixed fp16-stationary x fp8-moving matmul is exact on HW.  Tiles
  pack 2 groups so partition lines stay at the empirically best 6KB HWDGE
  descriptor size (~16GB/s per SDMA engine; 12KB and 24KB lines are
  slower, SWDGE much slower).
* Conv runs j-outer over group pairs with the block-diag weight padded to
  M=128 so the compiler enables Fast Weight Load on every matmul.
* exp(+bias) writes straight into per-chunk window tiles with row-sum
  accumulation (accum_out), DVE normalizes in place (1/Z per partition).
* The attn^T transposes run on the DMA XBAR (dma_start(transpose=True)).
  The scheduler gives XBAR-mode DMA an EXCLUSIVE window on the DMA channel
  (hw deadlock workaround), so each window's cost adds to the wall clock:
  the kernel uses 3 uneven l-chunks (12/12/8 groups) so windows 0-1 hide
  under the next chunk's compute and only the smallest window + tail is
  exposed at the end.  Score DMA issue runs 4 pairs ahead of consumption
  so the next chunk's tiles are emitted BEFORE each xbar trigger --
  otherwise they'd serialize behind the exclusive window and conv would
  starve through it.
* attn@V runs 2 heads concurrently in disjoint PE column groups
  (tile_position=(0,0)/(0,64)) accumulating into one [128, LCH] PSUM
  bank, making the PSUM->SBUF cast per head-pair a single tensor_copy
  whose partition layout exactly matches the row-parallel proj lhsT
  blocks.  Proj bias folds into PSUM via a ones-row matmul.
* Engine duty: Sync ring = score stream + xbar triggers (a transpose
  trigger blocks its issuing engine for the whole transfer, so it must
  not sit on ScalarE); Scalar ring = init loads + output stores; ScalarE
  compute = exp only; DVE = normalize/reciprocal/casts.
"""

import os
import sys

import numpy as np

sys.path.insert(0, "/opt/trn_rl_repo")

B, H, L, T, D = 2, 12, 1024, 1024, 64
DIM = H * D  # 768
NCORES = 8
LC = L * B // NCORES  # 256 l-rows per core
G = 8  # l-rows per conv group
NG = LC // G  # 32 groups
KM = 12 * G  # 96: conv matmul K and M
NTT = T // 128  # 8 t-tiles
# uneven l-chunks: each chunk's XBAR window hides under the next chunk's
# compute; the last (smallest) chunk minimizes the exposed window + tail
CS = [12, 12, 8]  # groups per chunk
NCH = len(CS)
CO = [sum(CS[:i]) for i in range(NCH)]  # group offsets
LCHS = [cs * G for cs in CS]  # l-rows per chunk (96, 96, 64)
LO = [co * G for co in CO]  # l-row offsets

_CACHE = {}


def _build_nc():
    import concourse.bacc as bacc
    import concourse.bass as bass
    import concourse.mybir as mybir
    import concourse.tile as tile
    from concourse.masks import make_identity
    from contextlib import ExitStack

    f32 = mybir.dt.float32
    f16 = mybir.dt.float16
    f8 = mybir.dt.float8e4

    nc = bacc.Bacc(
        "TRN2", target_bir_lowering=False, debug=False, enable_asserts=False
    )

    # host-packed inputs (scores fp8, rest fp16 except biases/output)
    # scores: fp8e4m3 (error budget: adds ~1.2e-2 absmax-rel vs the 2e-2
    # gate; conv weights STAY fp16 — mixed fp16-stationary x fp8-moving
    # matmul verified exact on HW), packed 2 groups per tile so partition
    # lines stay at the empirically best 6KB descriptor size.
    sp_in = nc.dram_tensor("sp", [NG // 2, KM, 6 * T], f8, kind="ExternalInput").ap()
    vp_in = nc.dram_tensor("vp", [128, H * NTT * D], f16, kind="ExternalInput").ap()
    # conv weights padded to M=128 (32 dummy output rows) so the compiler
    # enables Fast Weight Load (NumWeights==128) on every conv matmul
    wp_in = nc.dram_tensor("wp", [KM, 3 * 128], f16, kind="ExternalInput").ap()
    b_in = nc.dram_tensor("b96", [KM, 1], f32, kind="ExternalInput").ap()
    pw_in = nc.dram_tensor("pwp", [128, 6 * DIM], f16, kind="ExternalInput").ap()
    pb_in = nc.dram_tensor("pbr", [1, DIM], f16, kind="ExternalInput").ap()
    out_d = nc.dram_tensor("out", [LC, DIM], f16, kind="ExternalOutput").ap()

    with tile.TileContext(nc) as tc, ExitStack() as ctx:
        # ---- persistent SBUF ----
        singles = ctx.enter_context(tc.tile_pool(name="singles", bufs=1))
        ones1 = singles.tile([1, 128], f16)
        nc.vector.memset(ones1[:], 1.0)
        # only what the first conv+exp needs loads up front; the big vsb/pw
        # loads are issued later (phase 1) so they don't delay the first
        # score tiles on the shared SDMA engines
        wsb = singles.tile([KM, 3 * 128], f16)
        nc.scalar.dma_start(wsb[:], wp_in)
        b96 = singles.tile([KM, 1], f32)
        nc.scalar.dma_start(b96[:], b_in)
        pbr = singles.tile([1, DIM], f16)
        nc.scalar.dma_start(pbr[:], pb_in)
        vsb = singles.tile([128, H * NTT * D], f16)  # [t-part, h*512 + tt*64 + d]
        pw = singles.tile([128, 6 * DIM], f16)  # [i-part, ki*768 + o]
        # per-chunk attn^T: [t%128-part(128), gr*(NTT*96) + tt*96 + (h*8+lg)]
        # holds attn[l=LO[c] + gr*8 + lg, t=tt*128+p] for head h.
        # gr-major so the chunk's XBAR transpose writes one contiguous block.
        attnT = [
            singles.tile(
                [128, CS[c] * NTT * KM], f16, tag=f"attnT{c}", name=f"attnT{c}"
            )
            for c in range(NCH)
        ]
        # per-chunk x^T for proj: [i%128 part, (i//128)*LCHS[c] + l]
        xT = [
            singles.tile([128, 6 * LCHS[c]], f16, tag=f"xT{c}", name=f"xT{c}")
            for c in range(NCH)
        ]

        with ExitStack() as p1:
            spool = p1.enter_context(tc.tile_pool(name="scores", bufs=6))
            fpsum = p1.enter_context(tc.tile_pool(name="fpsum", bufs=2, space="PSUM"))
            npool = p1.enter_context(tc.tile_pool(name="norm", bufs=2))
            zpool = p1.enter_context(tc.tile_pool(name="z", bufs=4))
            xpsum = p1.enter_context(tc.tile_pool(name="xpsum", bufs=2, space="PSUM"))
            ppsum = p1.enter_context(tc.tile_pool(name="ppsum", bufs=1, space="PSUM"))
            ypool = p1.enter_context(tc.tile_pool(name="y", bufs=2))

            # Score-DMA issue runs LOOKAHEAD pairs ahead of consumption so
            # the next chunk's tiles are emitted BEFORE each chunk's xbar
            # trigger: otherwise they queue behind the ~17us exclusive
            # transpose window (in the Sync FIFO and in the scheduler's DMA
            # serialization order) and conv starves during the window.
            LA_PAIRS = 4  # 8 groups of lookahead
            st2_tiles = {}

            def issue_sp(p):
                if p >= NG // 2:
                    return
                st2 = spool.tile([KM, 6 * T], f8, tag="st2", name=f"st2_{p}")
                nc.sync.dma_start(st2[:], sp_in[p])
                st2_tiles[p] = st2

            for p in range(LA_PAIRS):
                issue_sp(p)

            for c in range(NCH):
                # ---- phase 1: conv + softmax, per pair of groups ----
                # (j-outer over the pair so identical conv weights are
                # consecutive on the PE queue)
                for gp in range(CS[c] // 2):
                    kp = CO[c] // 2 + gp  # global pair index
                    g0 = CO[c] + 2 * gp
                    issue_sp(kp + LA_PAIRS)
                    st2 = st2_tiles.pop(kp)
                    fps = []
                    for gi in range(2):
                        fps.append(
                            fpsum.tile([128, T], f32, tag="fp", name=f"fp_{g0 + gi}")
                        )
                    if kp == 5:
                        # big init loads, once the pipeline is rolling
                        nc.scalar.dma_start(vsb[:], vp_in)
                        nc.scalar.dma_start(pw[:], pw_in)
                    for j in range(3):
                        for gi in range(2):
                            for th in range(2):
                                nc.tensor.matmul(
                                    fps[gi][:, th * 512 : (th + 1) * 512],
                                    wsb[:, j * 128 : (j + 1) * 128],
                                    st2[
                                        :,
                                        gi * 3 * T
                                        + j * T
                                        + th * 512 : gi * 3 * T
                                        + j * T
                                        + (th + 1) * 512,
                                    ],
                                    start=(j == 0),
                                    stop=(j == 2),
                                )
                    for gi in range(2):
                        g = g0 + gi
                        q = g - CO[c]  # position in this chunk's window
                        if q == 0:
                            etn = npool.tile(
                                [KM, CS[c] * T], f16, tag="etn", name=f"etn_{g}"
                            )
                        # exp (+bias) straight into the window tile, with
                        # row-sum accumulation; then normalize in place
                        ecol = etn[:, q * T : (q + 1) * T]
                        zt = zpool.tile([KM, 1], f32, tag="zt")
                        nc.scalar.activation(
                            ecol,
                            fps[gi][0:KM, :],
                            mybir.ActivationFunctionType.Exp,
                            bias=b96[:],
                            accum_out=zt[:],
                        )
                        zi = zpool.tile([KM, 1], f32, tag="zi")
                        nc.vector.reciprocal(zi[:], zt[:])
                        nc.vector.tensor_scalar_mul(ecol, ecol, zi[:])
                        if q == CS[c] - 1:
                            # DMA XBAR transpose of the chunk's window:
                            # out[p, k, m] = etn[m, k*128+p], k = (gr, tt)
                            # (out must stay 3D: middle dim extends partition)
                            dst = attnT[c][:].rearrange("p (k m) -> p k m", m=KM)[
                                :, 0 : CS[c] * NTT, :
                            ]
                            nc.sync.dma_start(dst, etn[:], transpose=True)

                # ---- phase 2: attn @ V -> x^T, 2 heads per PSUM bank ----
                LCH = LCHS[c]
                atv = attnT[c][:].rearrange(
                    "p (gr tt h lg) -> p gr tt h lg", gr=CS[c], tt=NTT, h=H
                )
                for hp in range(H // 2):
                    h0, h1 = 2 * hp, 2 * hp + 1
                    xp = xpsum.tile([128, LCH], f32, tag="xp", name=f"xp_{c}_{hp}")
                    for tt in range(NTT):
                        nc.tensor.matmul(
                            xp[0:64, :],
                            vsb[:, h0 * 512 + tt * D : h0 * 512 + (tt + 1) * D],
                            atv[:, :, tt, h0, :],
                            start=(tt == 0),
                            stop=(tt == NTT - 1),
                            tile_position=(0, 0),
                        )
                        nc.tensor.matmul(
                            xp[64:128, :],
                            vsb[:, h1 * 512 + tt * D : h1 * 512 + (tt + 1) * D],
                            atv[:, :, tt, h1, :],
                            start=(tt == 0),
                            stop=(tt == NTT - 1),
                            tile_position=(0, 64),
                        )
                    nc.vector.tensor_copy(
                        xT[c][:, hp * LCH : (hp + 1) * LCH], xp[:]
                    )

                # ---- phase 3: proj (+bias via ones-row matmul) -> out ----
                pp = ppsum.tile([128, DIM], f32, tag="pp", name=f"pp_{c}")
                nc.tensor.matmul(
                    pp[0:LCH, 0:512],
                    ones1[:, 0:LCH],
                    pbr[:, 0:512],
                    start=True,
                    stop=False,
                )
                nc.tensor.matmul(
                    pp[0:LCH, 512:768],
                    ones1[:, 0:LCH],
                    pbr[:, 512:768],
                    start=True,
                    stop=False,
                )
                for ki in range(6):
                    lhs = xT[c][:, ki * LCH : (ki + 1) * LCH]
                    nc.tensor.matmul(
                        pp[0:LCH, 0:512],
                        lhs,
                        pw[:, ki * DIM : ki * DIM + 512],
                        start=False,
                        stop=(ki == 5),
                    )
                    nc.tensor.matmul(
                        pp[0:LCH, 512:768],
                        lhs,
                        pw[:, ki * DIM + 512 : ki * DIM + DIM],
                        start=False,
                        stop=(ki == 5),
                    )
                yt = ypool.tile([128, DIM], f16, tag="yt", name=f"yt_{c}")
                nc.vector.tensor_copy(yt[0:LCH, :], pp[0:LCH, :])
                nc.scalar.dma_start(out_d[LO[c] : LO[c] + LCH, :], yt[0:LCH, :])

    nc.compile()
    return nc


def _host_prep(s0, s1, s2, v, fuse_w, fuse_b, proj_w, proj_b):
    """Build per-core input maps (fp16 repack in exactly-SBUF layouts)."""
    s0 = np.asarray(s0, dtype=np.float32)
    s1 = np.asarray(s1, dtype=np.float32)
    s2 = np.asarray(s2, dtype=np.float32)
    v = np.asarray(v, dtype=np.float32)
    fuse_w = np.asarray(fuse_w, dtype=np.float32)
    fuse_b = np.asarray(fuse_b, dtype=np.float32)
    proj_w = np.asarray(proj_w, dtype=np.float32)
    proj_b = np.asarray(proj_b, dtype=np.float32)

    # block-diag conv weights: wp[k=(lg,c), j, m=(o,lg)] = fuse_w[o, 12j+c]
    # (padded to M=128 columns per j-block to enable FWL)
    wp = np.zeros((KM, 3, 128), dtype=np.float16)
    for j in range(3):
        blk = fuse_w[:, 12 * j : 12 * (j + 1)].T.astype(np.float16)  # [c, o]
        for lg in range(G):
            wp[lg * 12 : (lg + 1) * 12, j, lg : KM : G] = blk
    wp = wp.reshape(KM, 3 * 128)
    b96 = np.repeat(fuse_b, G).astype(np.float32).reshape(KM, 1)  # p = o*G+lg
    # proj weight chunks: pwp[p, ki, o] = proj_w[o, ki*128+p]
    pwp = (
        np.ascontiguousarray(proj_w.T.reshape(6, 128, DIM))
        .transpose(1, 0, 2)
        .reshape(128, 6 * DIM)
        .astype(np.float16)
    )
    pbr = proj_b.reshape(1, DIM).astype(np.float16)

    in_maps = []
    for k in range(NCORES):
        b = k // (NCORES // B)
        l0 = (k % (NCORES // B)) * LC
        # scores: sp[g, p=(lg*12+c), j, t] = s_j[b, c, l0+g*8+lg, t]
        sj = np.stack(
            [s[b, :, l0 : l0 + LC, :] for s in (s0, s1, s2)], axis=0
        )  # [3, 12, 256, 1024]
        import ml_dtypes

        sp = (
            sj.reshape(3, 12, NG, G, T)
            .transpose(2, 3, 1, 0, 4)  # [g, lg, c, j, t]
            .reshape(NG, KM, 3 * T)
            .astype(ml_dtypes.float8_e4m3fn)
            # 2 groups per tile: [p_, part, (gi, j, t)] -> 6KB fp8 lines
            .reshape(NG // 2, 2, KM, 3 * T)
            .transpose(0, 2, 1, 3)
            .reshape(NG // 2, KM, 6 * T)
        )
        # v: vp[p, h, tt, d] = v[b, h, tt*128+p, d]
        vp = (
            np.ascontiguousarray(v[b])  # [12, 1024, 64]
            .reshape(H, NTT, 128, D)
            .transpose(2, 0, 1, 3)
            .reshape(128, H * NTT * D)
            .astype(np.float16)
        )
        m = {
            "sp": np.ascontiguousarray(sp),
            "vp": np.ascontiguousarray(vp),
            "wp": wp,
            "b96": b96,
            "pwp": pwp,
            "pbr": pbr,
        }
        in_maps.append(m)
    return in_maps


def _install_ntff_hook():
    """Provide antenv.axon_hooks (absent in this image) so trace=True works."""
    try:
        from antenv import axon_hooks  # noqa: F401

        return True
    except ImportError:
        pass
    try:
        import types
        import ctypes
        import contextlib
        import antenv

        so_path = "/opt/axon/libaxon_pjrt.so"
        if not os.path.exists(so_path):
            return False
        lib = ctypes.CDLL(so_path)
        if not hasattr(lib, "axon_start_nrt_profile"):
            return False
        lib.axon_start_nrt_profile.argtypes = [
            ctypes.POINTER(ctypes.c_int64),
            ctypes.c_size_t,
        ]
        lib.axon_start_nrt_profile.restype = ctypes.c_int64
        lib.axon_stop_nrt_profile.argtypes = [ctypes.c_char_p]
        lib.axon_stop_nrt_profile.restype = ctypes.c_int64

        @contextlib.contextmanager
        def _hook(output_dir, device_ids):
            import jax

            jax.devices()
            if device_ids:
                ids = (ctypes.c_int64 * len(device_ids))(*device_ids)
                rc = lib.axon_start_nrt_profile(ids, len(device_ids))
            else:
                rc = lib.axon_start_nrt_profile(None, 0)
            if rc != 0:
                raise RuntimeError(f"axon_start_nrt_profile rc={rc}")
            try:
                yield
            finally:
                n = lib.axon_stop_nrt_profile(str(output_dir).encode())
                print(f"ntff profile: {n} file(s) -> {output_dir}", file=sys.stderr)

        mod = types.ModuleType("antenv.axon_hooks")
        _h = {"hook": _hook}
        mod.set_axon_ntff_profile_hook = lambda h: _h.__setitem__("hook", h)
        mod.get_axon_ntff_profile_hook = lambda: _h["hook"]
        sys.modules["antenv.axon_hooks"] = mod
        antenv.axon_hooks = mod
        return True
    except Exception as e:  # degrade to untraced
        print("ntff hook install failed:", e, file=sys.stderr)
        return False


def kernel(s0, s1, s2, v, fuse_w, fuse_b, proj_w, proj_b, _trace=False):
    from concourse import bass_utils
    from concourse.bass_utils import run_bass_kernel_spmd

    if "nc" not in _CACHE:
        _CACHE["nc"] = _build_nc()
    nc = _CACHE["nc"]

    in_maps = _host_prep(s0, s1, s2, v, fuse_w, fuse_b, proj_w, proj_b)
    if _trace:
        _trace = _install_ntff_hook()
        bass_utils.upload_artifacts = lambda tmpdir: f"local:{tmpdir}"
    tmpdir = None
    if _trace:
        import tempfile

        tmpdir = tempfile.mkdtemp(prefix="bass_trace_")
        _CACHE["trace_dir"] = tmpdir
    try:
        res = run_bass_kernel_spmd(
            nc, in_maps, core_ids=list(range(NCORES)), trace=_trace, tmpdir=tmpdir
        )
    except Exception:
        if not _trace:
            raise
        import traceback

        traceback.print_exc()
        print("trace run failed; retrying untraced", file=sys.stderr)
        res = run_bass_kernel_spmd(nc, in_maps, core_ids=list(range(NCORES)))
    _CACHE["last_exec_time_ns"] = res.exec_time_ns
    _CACHE["last_results"] = res

    out = np.empty((B, L, DIM), dtype=np.float32)
    for k in range(NCORES):
        b = k // (NCORES // B)
        l0 = (k % (NCORES // B)) * LC
        out[b, l0 : l0 + LC, :] = res.results[k]["out"].astype(np.float32)
    return out
